# revision 1
# baseline (speedup 1.0000x reference)
"""TRN2 Bass kernel for nn_CustomQLoRABigNet: 6 blocks x (3 QLoRA linears),
ReLU, residual, LayerNorm. Data-parallel over 8 NeuronCores (4096 rows each).

On-chip layout: hidden state kept feature-major ("transposed", [k, n]) so all
18 matmuls chain with contraction along SBUF partitions. Weights are
dequantized on-chip per layer: w = (q - 8) * scale, with q shipped as
pre-transposed centered int8 and scales shipped pre-replicated to the matching
[k, o] tile layout (host does pure layout transforms only). Matmuls run in
float32r (fp32 with 12-bit mantissa, full PE rate at N>=256).
"""

import sys

sys.path.insert(0, "/opt/trn_rl_repo")

import numpy as np
import ml_dtypes

import concourse.bass as bass
from concourse import bacc, mybir
import concourse.tile as tile
from concourse.bass_utils import run_bass_kernel_spmd

f32 = mybir.dt.float32
f32r = mybir.dt.float32r
i8 = mybir.dt.int8
bf16 = mybir.dt.bfloat16
AF = mybir.ActivationFunctionType
Alu = mybir.AluOpType

N_CORES = 8
DIM = 1024
KT = 8  # 1024 / 128 partition tiles
NL = 18
RANK = 32
GROUP = 16
BATCH = 32768
RPC = BATCH // N_CORES  # rows per core
CHUNK = 1024  # columns (rows of x) processed per weight pass
NT = 512  # matmul moving free dim (one PSUM bank)
EPS = 1e-5


def fp32r_round(a: np.ndarray) -> np.ndarray:
    """Round-to-nearest-even fp32 -> fp32r (low 12 mantissa bits cleared)."""
    u = np.ascontiguousarray(a, dtype=np.float32).view(np.uint32)
    low = u & np.uint32(0xFFF)
    base = u & ~np.uint32(0xFFF)
    lsb = (u >> np.uint32(12)) & np.uint32(1)
    up = (low > 0x800) | ((low == 0x800) & (lsb == 1))
    out = base + np.where(up, np.uint32(0x1000), np.uint32(0)).astype(np.uint32)
    return out.view(np.float32)


def build_kernel(rows_per_core: int = RPC, chunk: int = CHUNK, n_layers: int = NL):
    nc = bacc.Bacc()
    n_chunks = rows_per_core // chunk
    ntiles = chunk // NT
    n_blocks = n_layers // 3

    x_d = nc.declare_dram_parameter("x_t", [128, KT, rows_per_core], f32r, False)
    wq_d = nc.declare_dram_parameter("wqc", [n_layers, 128, KT, DIM], i8, False)
    sr_d = nc.declare_dram_parameter("srep", [n_layers, 128, KT, DIM], f32, False)
    la_d = nc.declare_dram_parameter("la_t", [n_layers, 128, KT, RANK], f32r, False)
    lb_d = nc.declare_dram_parameter("lb_t", [n_layers, 128, DIM], f32r, False)
    bi_d = nc.declare_dram_parameter("bias_pp", [128, n_layers, KT], f32, False)
    ga_d = nc.declare_dram_parameter("gamma_pp", [128, 5, KT], f32, False)
    be_d = nc.declare_dram_parameter("beta_pp", [128, 5, KT], f32, False)
    id_d = nc.declare_dram_parameter("ident", [128, 128], f32r, False)
    on_d = nc.declare_dram_parameter("ones", [128, 128], f32r, False)
    z_d = nc.declare_dram_parameter("zeros", [128, 2, NT], f32r, False)
    y_d = nc.declare_dram_parameter("y_t", [128, KT, rows_per_core], f32r, True)

    with tile.TileContext(nc) as tc:
        with (
            tc.tile_pool(name="persist", bufs=1) as pp,
            tc.tile_pool(name="wts", bufs=2) as wp,
            tc.tile_pool(name="work", bufs=2) as sp,
            tc.tile_pool(name="ps", bufs=1, space="PSUM") as ps,
        ):
            # persistent tiles
            h_a = pp.tile([128, KT, chunk], f32r)
            h_b = pp.tile([128, KT, chunk], f32r)
            r_t = pp.tile([128, KT, chunk], f32r)
            bias_t = pp.tile([128, n_layers, KT], f32)
            nc.sync.dma_start(bias_t[:, :, :], bi_d[:, :, :])
            gamma_t = pp.tile([128, 5, KT], f32)
            nc.sync.dma_start(gamma_t[:, :, :], ga_d[:, :, :])
            beta_t = pp.tile([128, 5, KT], f32)
            nc.sync.dma_start(beta_t[:, :, :], be_d[:, :, :])
            ident_t = pp.tile([128, 128], f32r)
            nc.sync.dma_start(ident_t[:, :], id_d[:, :])
            ones_t = pp.tile([128, 128], f32r)
            nc.sync.dma_start(ones_t[:, :], on_d[:, :])
            t_pad = pp.tile([128, 2, NT], f32r)
            nc.sync.dma_start(t_pad[:, :, :], z_d[:, :, :])
            ones_col = ones_t[:, 0:1]
            ones_row = ones_t[0:1, :]

            for c in range(n_chunks):
                ccols = bass.ts(c, chunk)
                h_x = h_a if c % 2 == 0 else h_b
                for kt in range(KT):
                    nc.sync.dma_start(h_x[:, kt, :], x_d[:, kt, ccols])
                nc.vector.tensor_copy(r_t[:, :, :], h_x[:, :, :])

                for l in range(n_layers):
                    blk, j = l // 3, l % 3
                    h_in = h_a if (l + c) % 2 == 0 else h_b
                    h_out = h_b if (l + c) % 2 == 0 else h_a

                    # ---- weight load + dequant ----
                    w_t = wp.tile([128, KT, DIM], f32r, tag="wt")
                    for kt in range(KT):
                        wq_t = wp.tile([128, DIM], i8, tag="wqc")
                        nc.sync.dma_start(wq_t[:, :], wq_d[l, :, kt, :])
                        sr_t = wp.tile([128, DIM], f32, tag="srep")
                        nc.sync.dma_start(sr_t[:, :], sr_d[l, :, kt, :])
                        nc.vector.tensor_mul(w_t[:, kt, :], wq_t[:, :], sr_t[:, :])
                    la_t = wp.tile([128, KT, RANK], f32r, tag="lat")
                    nc.sync.dma_start(la_t[:, :, :], la_d[l, :, :, :])
                    lb_t = wp.tile([128, DIM], f32r, tag="lbt")
                    nc.sync.dma_start(lb_t[:, :], lb_d[l, :, :])

                    for nt in range(ntiles):
                        cols = bass.ts(nt, NT)
                        # ---- LoRA stage 1: t = la @ h ----
                        t_ps = ps.tile([32, NT], f32, tag="t", bufs=2)
                        for kt in range(KT):
                            nc.tensor.matmul(
                                t_ps[:, :],
                                lhsT=la_t[:, kt, :],
                                rhs=h_in[:, kt, cols],
                                start=(kt == 0),
                                stop=(kt == KT - 1),
                            )
                        tbuf = (l * ntiles + nt) % 2
                        nc.vector.tensor_copy(t_pad[0:32, tbuf, :], t_ps[:, :])

                        # ---- base + lora stage2 + residual, per output tile ----
                        ln_here = j == 2 and blk < n_blocks - 1
                        if ln_here:
                            s1p = ps.tile([1, NT], f32, tag="s1", bufs=1)
                            s2p = ps.tile([1, NT], f32, tag="s2", bufs=1)
                        for ot in range(KT):
                            y_ps = ps.tile([128, NT], f32, tag="y", bufs=2)
                            for kt in range(KT):
                                nc.tensor.matmul(
                                    y_ps[:, :],
                                    lhsT=w_t[:, kt, bass.ts(ot, 128)],
                                    rhs=h_in[:, kt, cols],
                                    start=(kt == 0),
                                    stop=False,
                                )
                            nc.tensor.matmul(
                                y_ps[:, :],
                                lhsT=lb_t[:, bass.ts(ot, 128)],
                                rhs=t_pad[:, tbuf, :],
                                start=False,
                                stop=(j != 2),
                            )
                            if j == 2:
                                nc.tensor.matmul(
                                    y_ps[:, :],
                                    lhsT=ident_t[:, :],
                                    rhs=r_t[:, ot, cols],
                                    start=False,
                                    stop=True,
                                )
                            nc.scalar.activation(
                                h_out[:, ot, cols],
                                y_ps[:, :],
                                AF.Relu if j < 2 else AF.Identity,
                                bias=bias_t[:, l, ot : ot + 1],
                            )
                            if ln_here:
                                hsq = sp.tile([128, NT], f32r, tag="hsq")
                                nc.scalar.activation(
                                    hsq[:, :], h_out[:, ot, cols], AF.Square
                                )
                                nc.tensor.matmul(
                                    s1p[:, :], lhsT=ones_col,
                                    rhs=h_out[:, ot, cols],
                                    start=(ot == 0), stop=(ot == KT - 1),
                                )
                                nc.tensor.matmul(
                                    s2p[:, :], lhsT=ones_col, rhs=hsq[:, :],
                                    start=(ot == 0), stop=(ot == KT - 1),
                                )

                        # ---- LayerNorm at block end (blocks 0..4) ----
                        if ln_here:
                            m_sb = sp.tile([1, NT], f32, tag="m", bufs=1)
                            nc.vector.tensor_scalar(
                                m_sb[:, :], s1p[:, :], 1.0 / DIM, None, Alu.mult
                            )
                            var_sb = sp.tile([1, NT], f32, tag="var", bufs=1)
                            nc.vector.tensor_scalar(
                                var_sb[:, :], s2p[:, :], 1.0 / DIM, EPS,
                                Alu.mult, Alu.add,
                            )
                            msq = sp.tile([1, NT], f32, tag="msq", bufs=1)
                            nc.vector.tensor_mul(msq[:, :], m_sb[:, :], m_sb[:, :])
                            nc.vector.tensor_sub(var_sb[:, :], var_sb[:, :], msq[:, :])
                            lnv = sp.tile([1, NT], f32, tag="lnv", bufs=1)
                            nc.scalar.activation(lnv[:, :], var_sb[:, :], AF.Ln)
                            i_sb = sp.tile([1, NT], f32r, tag="isb", bufs=1)
                            nc.scalar.activation(i_sb[:, :], lnv[:, :], AF.Exp, scale=-0.5)
                            mi_sb = sp.tile([1, NT], f32r, tag="misb", bufs=1)
                            nc.vector.tensor_mul(mi_sb[:, :], m_sb[:, :], i_sb[:, :])
                            ib_ps = ps.tile([128, NT], f32, tag="bc", bufs=2)
                            nc.tensor.matmul(
                                ib_ps[:, :], lhsT=ones_row, rhs=i_sb[:, :],
                                start=True, stop=True,
                            )
                            mib_ps = ps.tile([128, NT], f32, tag="bc", bufs=2)
                            nc.tensor.matmul(
                                mib_ps[:, :], lhsT=ones_row, rhs=mi_sb[:, :],
                                start=True, stop=True,
                            )
                            for kt in range(KT):
                                nc.vector.tensor_mul(
                                    h_out[:, kt, cols], h_out[:, kt, cols], ib_ps[:, :]
                                )
                                nc.vector.tensor_sub(
                                    h_out[:, kt, cols], h_out[:, kt, cols], mib_ps[:, :]
                                )
                                nc.scalar.activation(
                                    h_out[:, kt, cols],
                                    h_out[:, kt, cols],
                                    AF.Identity,
                                    bias=beta_t[:, blk, kt : kt + 1],
                                    scale=gamma_t[:, blk, kt : kt + 1],
                                )
                                nc.vector.tensor_copy(
                                    r_t[:, kt, cols], h_out[:, kt, cols]
                                )

                h_fin = h_a if (n_layers + c) % 2 == 0 else h_b
                nc.sync.dma_start(y_d[:, :, ccols], h_fin[:, :, :])

    nc.compile()
    return nc


def prep_inputs(x, wq, scales, bias, lora_a, lora_b, gamma, beta,
                rows_per_core=RPC, n_layers=NL):
    """Host-side pure layout prep; returns per-core input maps."""
    nl = n_layers
    wqc = (wq[:nl].transpose(0, 2, 1).astype(np.int8) - 8)  # [l, k, o] centered
    wqc = wqc.reshape(nl, KT, 128, DIM).transpose(0, 2, 1, 3).copy()  # [l,p,kt,o]

    G = scales[:nl].reshape(nl, DIM, 64)  # [l, o, group]
    p_idx = np.arange(128)[:, None] // GROUP  # [128,1]
    kt_idx = np.arange(KT)[None, :] * (128 // GROUP)  # [1,8]
    gidx = p_idx + kt_idx  # [128, 8] -> group row index
    srep = G.transpose(0, 2, 1)[:, gidx, :].astype(np.float32).copy()  # [l,128,8,o]

    la_t = lora_a[:nl].transpose(0, 2, 1).reshape(nl, KT, 128, RANK)
    la_t = fp32r_round(la_t.transpose(0, 2, 1, 3)).copy()  # [l, p, kt, r]
    lb_small = fp32r_round(lora_b[:nl].transpose(0, 2, 1))  # [l, r, o]
    lb_t = np.zeros((nl, 128, DIM), np.float32)
    lb_t[:, :RANK, :] = lb_small

    bias_pp = bias[:nl].reshape(nl, KT, 128).transpose(2, 0, 1).astype(np.float32).copy()
    gamma_pp = gamma.reshape(5, KT, 128).transpose(2, 0, 1).astype(np.float32).copy()
    beta_pp = beta.reshape(5, KT, 128).transpose(2, 0, 1).astype(np.float32).copy()
    ident = np.eye(128, dtype=np.float32)

    shared = {
        "wqc": wqc, "srep": srep, "la_t": la_t, "lb_t": lb_t,
        "bias_pp": bias_pp, "gamma_pp": gamma_pp, "beta_pp": beta_pp,
        "ident": ident, "ones": np.ones((128, 128), np.float32),
        "zeros": np.zeros((128, 2, NT), np.float32),
    }
    in_maps = []
    for c in range(x.shape[0] // rows_per_core):
        xs = x[c * rows_per_core : (c + 1) * rows_per_core]  # [rows, 1024]
        x_t = fp32r_round(xs.T.reshape(KT, 128, rows_per_core).transpose(1, 0, 2)).copy()
        in_maps.append({"x_t": x_t, **shared})
    return in_maps


def unshard_output(results, rows_per_core=RPC):
    outs = []
    for r in results:
        y_t = np.asarray(r["y_t"]).reshape(128, KT, rows_per_core)
        outs.append(y_t.transpose(2, 1, 0).reshape(rows_per_core, DIM))
    return np.ascontiguousarray(np.concatenate(outs, axis=0), dtype=np.float32)


def kernel(x, wq, scales, bias, lora_a, lora_b, gamma, beta):
    x, wq, scales, bias, lora_a, lora_b, gamma, beta = (
        np.asarray(a) for a in (x, wq, scales, bias, lora_a, lora_b, gamma, beta)
    )
    nc = build_kernel()
    in_maps = prep_inputs(x, wq, scales, bias, lora_a, lora_b, gamma, beta)
    res = run_bass_kernel_spmd(nc, in_maps, list(range(N_CORES)))
    return unshard_output(res.results)



# revision 7
# speedup vs baseline: 1.3052x; 1.3052x over previous
"""v4: full fp16 hidden state + weights (FWL weight loads).

v3: w_eff dequant+fold once (chunk 0), cached in DRAM; LN apply on DVE.

TRN2 Bass kernel for nn_CustomQLoRABigNet: 6 blocks x (3 QLoRA linears),
ReLU, residual, LayerNorm. Data-parallel over 8 NeuronCores (4096 rows each).

v2 design:
- Hidden state kept feature-major [k, n] in f32r; all matmuls chain with
  contraction along SBUF partitions; fp32r runs at full PE rate at free dim
  512 (one PSUM bank).
- LoRA folded into the dequantized weight per layer: w_eff = (q-8)*s + lb@la
  (rank-32 PE matmuls into PSUM + DVE adds), removing both per-row LoRA
  matmuls from the row pipeline. Fold matmuls are interleaved into the base
  matmul stream so the PE never head-of-line blocks on the DVE adds.
- Three hidden-state buffers rotate (start buffer advances per chunk): the
  block input stays live for the residual with zero copies; the residual add
  + bias is fused into one DVE scalar_tensor_tensor eviction that updates the
  block-input buffer in place (per-output-slice, no hazard).
- j<2 layers pair the two 512-column strips per stationary weight tile
  (halves LDWEIGHTS pressure); j==2 layers run strips sequentially so the LN
  stat accumulators fit the PSUM bank budget (y0:2 y1:2 lo:2 s1:1 s2:1 = 8).
- LayerNorm: sums via PE ones-matmuls accumulated across output tiles;
  1/sigma via one scalar Abs_reciprocal_sqrt (keeps every scalar func in one
  act table set apart from it); inv and m*inv broadcast across partitions via
  PE; application h = h*inv - m*inv on the otherwise-idle GpSimd engine (two
  in-place tensor ops), then gamma/beta on the scalar engine per k-tile.
"""

import sys

sys.path.insert(0, "/opt/trn_rl_repo")

import numpy as np
import ml_dtypes

import concourse.bass as bass
from concourse import bacc, mybir
import concourse.tile as tile
from concourse.bass_utils import run_bass_kernel_spmd

f32 = mybir.dt.float32
f32r = mybir.dt.float32r
i8 = mybir.dt.int8
fp16 = mybir.dt.float16
AF = mybir.ActivationFunctionType
Alu = mybir.AluOpType

N_CORES = 8
DIM = 1024
KT = 8  # 1024 / 128 partition tiles
NL = 18
RANK = 32
BATCH = 32768
RPC = BATCH // N_CORES  # rows per core
CHUNK = 1024  # columns (rows of x) processed per weight pass
NT = 512  # matmul moving free dim (one PSUM bank)
EPS = 1e-5


def fp32r_round(a: np.ndarray) -> np.ndarray:
    """Round-to-nearest-even fp32 -> fp32r (low 12 mantissa bits cleared)."""
    u = np.ascontiguousarray(a, dtype=np.float32).view(np.uint32)
    low = u & np.uint32(0xFFF)
    base = u & ~np.uint32(0xFFF)
    lsb = (u >> np.uint32(12)) & np.uint32(1)
    up = (low > 0x800) | ((low == 0x800) & (lsb == 1))
    out = base + np.where(up, np.uint32(0x1000), np.uint32(0)).astype(np.uint32)
    return out.view(np.float32)


def build_kernel(rows_per_core: int = RPC, chunk: int = CHUNK, n_layers: int = NL):
    nc = bacc.Bacc()
    n_chunks = rows_per_core // chunk
    ntiles = chunk // NT
    assert ntiles == 2

    x_d = nc.declare_dram_parameter("x_t", [128, KT, rows_per_core], fp16, False)
    wq_d = nc.declare_dram_parameter("wqc", [n_layers, 128, KT, DIM], i8, False)
    sr_d = nc.declare_dram_parameter("srep", [n_layers, 128, KT, DIM], f32, False)
    la_d = nc.declare_dram_parameter("la_p", [n_layers, RANK, KT, 128], f32r, False)
    lb_d = nc.declare_dram_parameter("lb_t", [n_layers, RANK, DIM], f32r, False)
    bi_d = nc.declare_dram_parameter("bias_pp", [128, n_layers, KT], f32, False)
    ga_d = nc.declare_dram_parameter("gamma_pp", [128, 5, KT], f32, False)
    be_d = nc.declare_dram_parameter("beta_pp", [128, 5, KT], f32, False)
    on_d = nc.declare_dram_parameter("ones", [128, 128], fp16, False)
    y_d = nc.declare_dram_parameter("y_t", [128, KT, rows_per_core], fp16, True)
    wc_d = nc.dram_tensor("wcache", [n_layers, 128, KT, DIM], fp16, kind="Internal")

    with tile.TileContext(nc) as tc:
        with (
            tc.tile_pool(name="persist", bufs=1) as pp,
            tc.tile_pool(name="wts", bufs=2) as wp,
            tc.tile_pool(name="work", bufs=2) as sp,
            tc.tile_pool(name="ps", bufs=1, space="PSUM") as ps,
        ):
            # ---- persistent tiles ----
            hbuf = [
                pp.tile([128, KT, chunk], fp16, tag=f"h{i}", name=f"h{i}")
                for i in range(3)
            ]
            nc.sync.dma_start(hbuf[0][:, :, :], x_d[:, :, bass.ts(0, chunk)])
            pr0 = prep_dma_holder = {}
            bias_t = pp.tile([128, n_layers, KT], f32)
            nc.sync.dma_start(bias_t[:, :, :], bi_d[:, :, :])
            gamma_t = pp.tile([128, 5, KT], f32)
            nc.sync.dma_start(gamma_t[:, :, :], ga_d[:, :, :])
            beta_t = pp.tile([128, 5, KT], f32)
            nc.sync.dma_start(beta_t[:, :, :], be_d[:, :, :])
            ones_t = pp.tile([128, 128], fp16)
            nc.sync.dma_start(ones_t[:, :], on_d[:, :])
            ones_col = ones_t[:, 0:1]
            ones_row = ones_t[0:1, :]

            def prep_dma(l):
                """DMA weight raw material for layer l (into wts pool)."""
                wq_ts, sr_ts = [], []
                for kt in range(KT):
                    wq_t = wp.tile([128, DIM], i8, tag="wq", bufs=3)
                    nc.sync.dma_start(wq_t[:, :], wq_d[l, :, kt, :])
                    sr_t = wp.tile([128, DIM], f32, tag="sr", bufs=3)
                    nc.sync.dma_start(sr_t[:, :], sr_d[l, :, kt, :])
                    wq_ts.append(wq_t)
                    sr_ts.append(sr_t)
                la_t = wp.tile([RANK, KT, 128], f32r, tag="la", bufs=1)
                nc.sync.dma_start(la_t[:, :, :], la_d[l, :, :, :])
                lb_t = wp.tile([RANK, DIM], f32r, tag="lb", bufs=1)
                nc.sync.dma_start(lb_t[:, :], lb_d[l, :, :])
                return wq_ts, sr_ts, la_t, lb_t

            def prep_mul(pr, w_t):
                """Dequant multiply on DVE: w_t[:, kt, :] = wq * srep."""
                wq_ts, sr_ts, _, _ = pr
                for kt in range(KT):
                    nc.vector.tensor_mul(w_t[:, kt, :], wq_ts[kt][:, :], sr_ts[kt][:, :])

            def fold_mm(pr, kt):
                """PE: lora outer product (lb@la).T for one kt into 2 PSUM halves."""
                _, _, la_t, lb_t = pr
                halves = []
                for h in range(2):
                    t = ps.tile([128, NT], f32, tag="lo", bufs=2)
                    nc.tensor.matmul(
                        t[:, :], lhsT=la_t[:, kt, :],
                        rhs=lb_t[:, bass.ts(h, NT)],
                        start=True, stop=True,
                    )
                    halves.append(t)
                return halves

            def fold_add(halves, w_t, kt):
                """DVE: w_t[:, kt, :] += lora psum halves."""
                for h in range(2):
                    nc.vector.tensor_add(
                        w_t[:, kt, bass.ts(h, NT)],
                        w_t[:, kt, bass.ts(h, NT)],
                        halves[h][:, :],
                    )

            for c in range(n_chunks):
                s = c % 3
                ccols = bass.ts(c, chunk)
                if c == 0:
                    pr = prep_dma(0)
                    w_cur = wp.tile([128, KT, DIM], fp16, tag="wt")
                    prep_mul(pr, w_cur)
                    for kt in range(KT):
                        lo = fold_mm(pr, kt)
                        fold_add(lo, w_cur, kt)
                    nc.sync.dma_start(wc_d[0, :, :, :], w_cur[:, :, :])

                for l in range(n_layers):
                    blk, j = l // 3, l % 3
                    h_in = hbuf[(s + l) % 3]
                    h_out = hbuf[(s + l + 1) % 3]
                    ln_here = j == 2 and blk < 5
                    last = c == n_chunks - 1 and l == n_layers - 1

                    # Weight prep for the next layer (wraps to l=0 for the
                    # next chunk's first layer). Chunk 0 builds w_eff from raw
                    # material and caches it to DRAM; later chunks (and the
                    # chunk-0 -> chunk-1 boundary) just DMA the cached w_eff.
                    build = c == 0 and l + 1 < n_layers
                    if not last:
                        w_nxt = wp.tile([128, KT, DIM], fp16, tag="wt")
                        if build:
                            pr_n = prep_dma(l + 1)
                        else:
                            nc.sync.dma_start(
                                w_nxt[:, :, :], wc_d[(l + 1) % n_layers, :, :, :]
                            )

                    if j < 2:
                        # ---- paired strips: one stationary tile, two mms ----
                        if build:
                            prep_mul(pr_n, w_nxt)
                        lo_pend = None
                        for ot in range(KT):
                            y0 = ps.tile([128, NT], f32, tag="y0", bufs=2)
                            y1 = ps.tile([128, NT], f32, tag="y1", bufs=2)
                            for kt in range(KT):
                                lhs = w_cur[:, kt, bass.ts(ot, 128)]
                                nc.tensor.matmul(
                                    y0[:, :], lhsT=lhs,
                                    rhs=h_in[:, kt, bass.ts(0, NT)],
                                    start=(kt == 0), stop=(kt == KT - 1),
                                )
                                nc.tensor.matmul(
                                    y1[:, :], lhsT=lhs,
                                    rhs=h_in[:, kt, bass.ts(1, NT)],
                                    start=(kt == 0), stop=(kt == KT - 1),
                                )
                            nc.scalar.activation(
                                h_out[:, ot, bass.ts(0, NT)], y0[:, :], AF.Relu,
                                bias=bias_t[:, l, ot : ot + 1],
                            )
                            nc.scalar.activation(
                                h_out[:, ot, bass.ts(1, NT)], y1[:, :], AF.Relu,
                                bias=bias_t[:, l, ot : ot + 1],
                            )
                            # interleave next layer's lora fold: mm at ot,
                            # DVE add one iteration later (keeps PE unblocked)
                            if build:
                                if lo_pend is not None:
                                    fold_add(lo_pend, w_nxt, ot - 1)
                                lo_pend = fold_mm(pr_n, ot)
                        if build:
                            fold_add(lo_pend, w_nxt, KT - 1)
                            nc.sync.dma_start(wc_d[l + 1, :, :, :], w_nxt[:, :, :])
                    else:
                        # ---- sequential strips with residual (+ LN stats) ----
                        # h_out is hbuf[s]: the block input (residual source),
                        # updated in place slice by slice.
                        for nt in range(ntiles):
                            cols = bass.ts(nt, NT)
                            if ln_here:
                                s1p = ps.tile([1, NT], f32, tag="s1", bufs=1)
                                s2p = ps.tile([1, NT], f32, tag="s2", bufs=1)
                            lo_pend = None
                            for ot in range(KT):
                                y0 = ps.tile([128, NT], f32, tag="y0", bufs=2)
                                for kt in range(KT):
                                    nc.tensor.matmul(
                                        y0[:, :],
                                        lhsT=w_cur[:, kt, bass.ts(ot, 128)],
                                        rhs=h_in[:, kt, cols],
                                        start=(kt == 0), stop=(kt == KT - 1),
                                    )
                                # fused eviction: (psum + bias) + residual
                                nc.vector.scalar_tensor_tensor(
                                    h_out[:, ot, cols], y0[:, :],
                                    bias_t[:, l, ot : ot + 1],
                                    h_out[:, ot, cols],
                                    Alu.add, Alu.add,
                                )
                                if ln_here:
                                    hsq = sp.tile([128, NT], fp16, tag="hsq")
                                    nc.scalar.activation(
                                        hsq[:, :], h_out[:, ot, cols], AF.Square
                                    )
                                    # accumulate k-tile partials on DVE
                                    if ot == 0:
                                        s1a = sp.tile([128, NT], fp16, tag="s1a")
                                        nc.vector.tensor_copy(
                                            s1a[:, :], h_out[:, ot, cols]
                                        )
                                        s2a = sp.tile([128, NT], fp16, tag="s2a")
                                        nc.vector.tensor_copy(s2a[:, :], hsq[:, :])
                                    else:
                                        nc.vector.tensor_add(
                                            s1a[:, :], s1a[:, :], h_out[:, ot, cols]
                                        )
                                        nc.vector.tensor_add(
                                            s2a[:, :], s2a[:, :], hsq[:, :]
                                        )
                                # interleave next layer's lora fold in strip 1
                                if nt == 1 and build:
                                    if lo_pend is not None:
                                        fold_add(lo_pend, w_nxt, ot - 1)
                                    lo_pend = fold_mm(pr_n, ot)
                            if nt == 0 and build:
                                prep_mul(pr_n, w_nxt)
                            if nt == 1 and build:
                                fold_add(lo_pend, w_nxt, KT - 1)
                                nc.sync.dma_start(
                                    wc_d[l + 1, :, :, :], w_nxt[:, :, :]
                                )

                            if ln_here:
                                nc.tensor.matmul(
                                    s1p[:, :], lhsT=ones_col, rhs=s1a[:, :],
                                    start=True, stop=True,
                                )
                                nc.tensor.matmul(
                                    s2p[:, :], lhsT=ones_col, rhs=s2a[:, :],
                                    start=True, stop=True,
                                )
                                # mean / var / 1/sigma chain ([1, NT] ops)
                                m_sb = sp.tile([1, NT], f32, tag="m", bufs=1)
                                nc.vector.tensor_scalar(
                                    m_sb[:, :], s1p[:, :], 1.0 / DIM, None, Alu.mult
                                )
                                var_sb = sp.tile([1, NT], f32, tag="var", bufs=1)
                                nc.vector.tensor_scalar(
                                    var_sb[:, :], s2p[:, :], 1.0 / DIM, EPS,
                                    Alu.mult, Alu.add,
                                )
                                msq = sp.tile([1, NT], f32, tag="msq", bufs=1)
                                nc.vector.tensor_mul(msq[:, :], m_sb[:, :], m_sb[:, :])
                                nc.vector.tensor_sub(
                                    var_sb[:, :], var_sb[:, :], msq[:, :]
                                )
                                inv_sb = sp.tile([1, NT], fp16, tag="inv", bufs=1)
                                nc.scalar.activation(
                                    inv_sb[:, :], var_sb[:, :], AF.Abs_reciprocal_sqrt
                                )
                                mi_sb = sp.tile([1, NT], fp16, tag="mi", bufs=1)
                                nc.vector.tensor_mul(
                                    mi_sb[:, :], m_sb[:, :], inv_sb[:, :]
                                )
                                # broadcast inv and -m*inv across partitions
                                ib_ps = ps.tile([128, NT], f32, tag="y1", bufs=2)
                                nc.tensor.matmul(
                                    ib_ps[:, :], lhsT=ones_row, rhs=inv_sb[:, :],
                                    start=True, stop=True,
                                )
                                mib_ps = ps.tile([128, NT], f32, tag="y1", bufs=2)
                                nc.tensor.matmul(
                                    mib_ps[:, :], lhsT=ones_row, rhs=mi_sb[:, :],
                                    start=True, stop=True,
                                )
                                # apply on DVE (reads PSUM broadcasts), then
                                # gamma/beta on scalar per k-tile
                                for kt in range(KT):
                                    nc.vector.tensor_mul(
                                        h_out[:, kt, cols], h_out[:, kt, cols],
                                        ib_ps[:, :],
                                    )
                                    nc.vector.tensor_sub(
                                        h_out[:, kt, cols], h_out[:, kt, cols],
                                        mib_ps[:, :],
                                    )
                                    nc.scalar.activation(
                                        h_out[:, kt, cols], h_out[:, kt, cols],
                                        AF.Identity,
                                        bias=beta_t[:, blk, kt : kt + 1],
                                        scale=gamma_t[:, blk, kt : kt + 1],
                                    )

                    if not last:
                        w_cur = w_nxt

                    # prefetch next chunk's x into its start buffer once its
                    # last reader (layer 16 input) has been emitted
                    if l == 16 and c + 1 < n_chunks:
                        nc.sync.dma_start(
                            hbuf[(s + 1) % 3][:, :, :],
                            x_d[:, :, bass.ts(c + 1, chunk)],
                        )

                nc.sync.dma_start(y_d[:, :, ccols], hbuf[s][:, :, :])

    nc.compile()
    return nc


def prep_inputs(x, wq, scales, bias, lora_a, lora_b, gamma, beta,
                rows_per_core=RPC, n_layers=NL):
    """Host-side pure layout prep; returns per-core input maps."""
    nl = n_layers
    wqc = (wq[:nl].transpose(0, 2, 1).astype(np.int8) - 8)  # [l, k, o] centered
    wqc = wqc.reshape(nl, KT, 128, DIM).transpose(0, 2, 1, 3).copy()  # [l,p,kt,o]

    G = scales[:nl].reshape(nl, DIM, 64)  # [l, o, group]
    p_idx = np.arange(128)[:, None] // 16  # [128,1]
    kt_idx = np.arange(KT)[None, :] * (128 // 16)  # [1,8]
    gidx = p_idx + kt_idx  # [128, 8] -> group row index
    srep = G.transpose(0, 2, 1)[:, gidx, :].astype(np.float32).copy()  # [l,128,8,o]

    # la_p[l, r, kt, p] = la[l, r, kt*128 + p]
    la_p = fp32r_round(lora_a[:nl].reshape(nl, RANK, KT, 128)).copy()
    lb_t = fp32r_round(lora_b[:nl].transpose(0, 2, 1)).copy()  # [l, r, o]

    bias_pp = bias[:nl].reshape(nl, KT, 128).transpose(2, 0, 1).astype(np.float32).copy()
    gamma_pp = gamma.reshape(5, KT, 128).transpose(2, 0, 1).astype(np.float32).copy()
    beta_pp = beta.reshape(5, KT, 128).transpose(2, 0, 1).astype(np.float32).copy()

    shared = {
        "wqc": wqc, "srep": srep, "la_p": la_p, "lb_t": lb_t,
        "bias_pp": bias_pp, "gamma_pp": gamma_pp, "beta_pp": beta_pp,
        "ones": np.ones((128, 128), np.float16),
    }
    in_maps = []
    for c in range(x.shape[0] // rows_per_core):
        xs = x[c * rows_per_core : (c + 1) * rows_per_core]  # [rows, 1024]
        x_t = np.ascontiguousarray(
            xs.T.reshape(KT, 128, rows_per_core).transpose(1, 0, 2)
        ).astype(np.float16)
        in_maps.append({"x_t": x_t, **shared})
    return in_maps


def unshard_output(results, rows_per_core=RPC):
    outs = []
    for r in results:
        y_t = np.asarray(r["y_t"]).astype(np.float32).reshape(128, KT, rows_per_core)
        outs.append(y_t.transpose(2, 1, 0).reshape(rows_per_core, DIM))
    return np.ascontiguousarray(np.concatenate(outs, axis=0), dtype=np.float32)


def kernel(x, wq, scales, bias, lora_a, lora_b, gamma, beta):
    x, wq, scales, bias, lora_a, lora_b, gamma, beta = (
        np.asarray(a) for a in (x, wq, scales, bias, lora_a, lora_b, gamma, beta)
    )
    nc = build_kernel()
    in_maps = prep_inputs(x, wq, scales, bias, lora_a, lora_b, gamma, beta)
    res = run_bass_kernel_spmd(nc, in_maps, list(range(N_CORES)))
    return unshard_output(res.results)
